# revision 1
# baseline (speedup 1.0000x reference)
"""HMM forward-backward marginal (nn_HMM_EM) on 8 Trainium2 NeuronCores.

Batch (8192) is sharded across 8 cores (1024 each); tiny T/pi/emit params are
replicated. Per core, in transposed (Z, B) layout:
  gather:  e_t^T = emitS^T @ O_t      (O_t = one-hot of tokens, K=64 matmul)
  recurse: beta^T <- T^T @ (e_t^T * beta^T)   11 steps, PSUM-accumulated
  reduce:  s = pi^T @ (e_0^T * beta^T)        (1, B) per core
Host post-processing: out = S*log(SCALE) - log(s).
Emissions are pre-scaled by SCALE=128 on the host so all intermediates stay
well inside fp32 range (log s ~ -55 + 12*log 128 ~ +3).
"""

import sys

sys.path.insert(0, "/opt/trn_rl_repo")

import numpy as np
import ml_dtypes

Z = 256        # hidden states
X = 64         # emission symbols
S = 12         # sequence length
B = 8192       # total batch
NCORES = 8
BL = B // NCORES   # 1024 batch per core
NBF = 512          # matmul free-dim chunk (one PSUM bank of fp32)
NB = BL // NBF     # 2 batch chunks per core
SCALE = 128.0

BF16 = ml_dtypes.bfloat16

_CACHE: dict = {}


def _build_bass():
    import concourse.mybir as mybir
    from concourse import bacc
    from concourse.tile import TileContext

    DT = mybir.dt.bfloat16
    F32 = mybir.dt.float32

    nc = bacc.Bacc("TRN2", target_bir_lowering=False, debug=False)

    O2 = nc.dram_tensor("O2", [128, (S // 2) * BL], DT, kind="ExternalInput")
    Tm = nc.dram_tensor("Tm", [Z, Z], DT, kind="ExternalInput")
    emit2 = nc.dram_tensor("emit2", [128, Z], DT, kind="ExternalInput")
    pi2 = nc.dram_tensor("pi2", [128, 2], DT, kind="ExternalInput")
    out_s = nc.dram_tensor("out_s", [1, BL], F32, kind="ExternalOutput")

    with TileContext(nc) as tc:
        with (
            tc.tile_pool(name="const", bufs=1) as const,
            tc.tile_pool(name="esb", bufs=6) as epool,
            tc.tile_pool(name="bsb", bufs=4) as bpool,
            tc.tile_pool(name="wsb", bufs=6) as wpool,
            tc.tile_pool(name="osb", bufs=2) as opool,
            tc.tile_pool(name="pse", bufs=4, space="PSUM") as pse,
            tc.tile_pool(name="psb", bufs=1, space="PSUM") as psb,
        ):
            T_sb = [const.tile([128, Z], DT, name=f"T{k}") for k in range(2)]
            emit_sb = const.tile([128, Z], DT, name="emit2")
            pi_sb = const.tile([128, 2], DT, name="pi2")
            O_sb = [const.tile([128, NBF], DT, name=f"O{j}") for j in range(S)]

            nc.sync.dma_start(out=emit_sb[:], in_=emit2[:])
            nc.sync.dma_start(out=pi_sb[:], in_=pi2[:])
            for k in range(2):
                nc.sync.dma_start(out=T_sb[k][:], in_=Tm[k * 128 : (k + 1) * 128, :])
            # O chunk j holds columns [j*NBF, (j+1)*NBF) of the packed one-hot;
            # chunk index for (t, bc) is (t//2)*NB + bc. DMA in use order.
            order: list[int] = []
            for t in range(S - 1, -1, -1):
                for bc in range(NB):
                    j = (t // 2) * NB + bc
                    if j not in order:
                        order.append(j)
            for j in order:
                nc.sync.dma_start(
                    out=O_sb[j][:], in_=O2[:, j * NBF : (j + 1) * NBF]
                )

            # persistent PSUM accumulators for beta^T, per (batch-chunk, z-chunk)
            beta_ps = [
                [psb.tile([128, NBF], F32, name=f"beta{bc}{m}") for m in range(2)]
                for bc in range(NB)
            ]

            for t in range(S - 1, -1, -1):
                for bc in range(NB):
                    j = (t // 2) * NB + bc
                    pr = slice(64 * (t % 2), 64 * (t % 2) + 64)

                    # gather e_t^T (two z-chunks) via one-hot matmul, K=64
                    e_ps = [pse.tile([128, NBF], F32, name="eps") for _ in range(2)]
                    for m in range(2):
                        nc.tensor.matmul(
                            e_ps[m][:],
                            emit_sb[pr, m * 128 : (m + 1) * 128],
                            O_sb[j][pr, :],
                            start=True,
                            stop=True,
                        )
                    e_sb = [epool.tile([128, NBF], DT, name="esb") for _ in range(2)]
                    for m in range(2):
                        nc.scalar.copy(out=e_sb[m][:], in_=e_ps[m][:])

                    if t == S - 1:
                        w = e_sb  # beta starts at ones
                    else:
                        b_sb = [
                            bpool.tile([128, NBF], DT, name="bsb") for _ in range(2)
                        ]
                        for m in range(2):
                            nc.scalar.copy(out=b_sb[m][:], in_=beta_ps[bc][m][:])
                        w = [wpool.tile([128, NBF], DT, name="wsb") for _ in range(2)]
                        for m in range(2):
                            nc.vector.tensor_mul(
                                out=w[m][:], in0=e_sb[m][:], in1=b_sb[m][:]
                            )

                    if t > 0:
                        for m in range(2):
                            for k in range(2):
                                nc.tensor.matmul(
                                    beta_ps[bc][m][:],
                                    T_sb[k][:, m * 128 : (m + 1) * 128],
                                    w[k][:],
                                    start=(k == 0),
                                    stop=(k == 1),
                                )
                    else:
                        s_ps = pse.tile([128, NBF], F32, name="eps")
                        for k in range(2):
                            nc.tensor.matmul(
                                s_ps[0:1, :],
                                pi_sb[:, k : k + 1],
                                w[k][:],
                                start=(k == 0),
                                stop=(k == 1),
                            )
                        s_sb = opool.tile([1, NBF], F32, name="ssb")
                        nc.vector.tensor_copy(out=s_sb[:], in_=s_ps[0:1, :])
                        nc.sync.dma_start(
                            out=out_s[0:1, bc * NBF : (bc + 1) * NBF], in_=s_sb[:]
                        )

    nc.compile()
    return nc


def _get_nc():
    if "nc" not in _CACHE:
        _CACHE["nc"] = _build_bass()
    return _CACHE["nc"]


def _softmax0(x):
    x = np.asarray(x, np.float32)
    m = x.max(axis=0, keepdims=True)
    e = np.exp(x - m)
    return e / e.sum(axis=0, keepdims=True)


def _prepare_in_maps(tokens, T_logits, pi_logits, emit_logits):
    tokens = np.asarray(tokens).astype(np.int32)
    T = _softmax0(T_logits)                      # (Z, Z) columns sum to 1
    pi = _softmax0(pi_logits)                    # (Z,)
    emit = _softmax0(emit_logits) * np.float32(SCALE)  # (X, Z), pre-scaled

    Tb = T.astype(BF16)
    emit2 = np.concatenate([emit, emit], axis=0).astype(BF16)   # (128, Z)
    pi2 = pi.reshape(2, 128).T.copy().astype(BF16)              # (128, 2)

    # Packed one-hot: O[core, 64*(t%2)+token, (t//2)*BL + b_local] = 1
    O = np.zeros((NCORES, 128, (S // 2) * BL), np.float32)
    b = np.arange(B)
    core = b // BL
    bl = b % BL
    for t in range(S):
        O[core, 64 * (t % 2) + tokens[t], (t // 2) * BL + bl] = 1.0
    O = O.astype(BF16)

    return [
        {"O2": O[c], "Tm": Tb, "emit2": emit2, "pi2": pi2}
        for c in range(NCORES)
    ]


def _run(inputs, trace=False, tmpdir=None):
    from concourse.bass_utils import run_bass_kernel_spmd

    in_maps = _prepare_in_maps(
        inputs["tokens"],
        inputs["T_logits"],
        inputs["pi_logits"],
        inputs["emit_logits"],
    )
    nc = _get_nc()
    res = run_bass_kernel_spmd(
        nc, in_maps, list(range(NCORES)), trace=trace, tmpdir=tmpdir
    )
    s = np.concatenate(
        [res.results[c]["out_s"].reshape(-1) for c in range(NCORES)]
    ).astype(np.float32)
    out = np.float32(S * np.log(SCALE)) - np.log(s)
    return out.astype(np.float32), res


def kernel(**inputs):
    return _run(inputs, trace=False)[0]


# revision 2
# speedup vs baseline: 1.2199x; 1.2199x over previous
"""HMM forward-backward marginal (nn_HMM_EM) on 8 Trainium2 NeuronCores.

Batch (8192) is sharded across 8 cores (1024 each); tiny T/pi/emit params are
replicated. Per core, in transposed (Z, B) layout:
  gather:  e_t^T = emitS^T @ O_t      (O_t = one-hot of tokens, K=64 matmul)
  recurse: beta^T <- T^T @ (e_t^T * beta^T)   11 steps, PSUM-accumulated
  reduce:  s = pi^T @ (e_0^T * beta^T)        (1, B) per core
Host post-processing: out = S*log(SCALE) - log(s).
Emissions are pre-scaled by SCALE=128 on the host so all intermediates stay
well inside fp32 range (log s ~ -55 + 12*log 128 ~ +3).
"""

import sys

sys.path.insert(0, "/opt/trn_rl_repo")

import numpy as np
import ml_dtypes

Z = 256        # hidden states
X = 64         # emission symbols
S = 12         # sequence length
B = 8192       # total batch
NCORES = 8
BL = B // NCORES   # 1024 batch per core
NBF = 512          # matmul free-dim chunk (one PSUM bank of fp32)
NB = BL // NBF     # 2 batch chunks per core
SCALE = 128.0

BF16 = ml_dtypes.bfloat16

_CACHE: dict = {}


def _build_bass():
    import concourse.mybir as mybir
    from concourse import bacc
    from concourse.tile import TileContext

    DT = mybir.dt.bfloat16
    F32 = mybir.dt.float32

    nc = bacc.Bacc("TRN2", target_bir_lowering=False, debug=False)

    O2 = nc.dram_tensor("O2", [128, (S // 2) * BL], DT, kind="ExternalInput")
    Tm = nc.dram_tensor("Tm", [Z, Z], DT, kind="ExternalInput")
    emit2 = nc.dram_tensor("emit2", [128, Z], DT, kind="ExternalInput")
    pi2 = nc.dram_tensor("pi2", [128, 2], DT, kind="ExternalInput")
    out_s = nc.dram_tensor("out_s", [1, BL], F32, kind="ExternalOutput")

    ZT = 2 * NBF  # 1024: two z-chunks side by side in the free dim

    with TileContext(nc) as tc:
        with (
            tc.tile_pool(name="const", bufs=1) as const,
            tc.tile_pool(name="esb", bufs=3) as epool,
            tc.tile_pool(name="wsb", bufs=4) as wpool,
            tc.tile_pool(name="osb", bufs=2) as opool,
            tc.tile_pool(name="pse", bufs=2, space="PSUM") as pse,
            tc.tile_pool(name="psb", bufs=1, space="PSUM") as psb,
        ):
            T_sb = [const.tile([128, Z], DT, name=f"T{k}") for k in range(2)]
            emit_sb = const.tile([128, Z], DT, name="emit2")
            pi_sb = const.tile([128, 2], DT, name="pi2")
            O_sb = [const.tile([128, NBF], DT, name=f"O{j}") for j in range(S)]

            nc.sync.dma_start(out=emit_sb[:], in_=emit2[:])
            nc.sync.dma_start(out=pi_sb[:], in_=pi2[:])
            for k in range(2):
                nc.sync.dma_start(out=T_sb[k][:], in_=Tm[k * 128 : (k + 1) * 128, :])
            # O chunk j holds columns [j*NBF, (j+1)*NBF) of the packed one-hot;
            # chunk index for (t, bc) is (t//2)*NB + bc. DMA in use order.
            order: list[int] = []
            for t in range(S - 1, -1, -1):
                for bc in range(NB):
                    j = (t // 2) * NB + bc
                    if j not in order:
                        order.append(j)
            for j in order:
                nc.sync.dma_start(
                    out=O_sb[j][:], in_=O2[:, j * NBF : (j + 1) * NBF]
                )

            # persistent 2-bank PSUM accumulators for beta^T, one per batch
            # chunk; columns [m*NBF,(m+1)*NBF) hold z-chunk m.
            beta_ps = [psb.tile([128, ZT], F32, name=f"beta{bc}") for bc in range(NB)]

            for t in range(S - 1, -1, -1):
                for bc in range(NB):
                    j = (t // 2) * NB + bc
                    pr = slice(64 * (t % 2), 64 * (t % 2) + 64)

                    # gather e_t^T (two z-chunks) via one-hot matmul, K=64
                    e_ps = pse.tile([128, ZT], F32, name="eps")
                    for m in range(2):
                        nc.tensor.matmul(
                            e_ps[:, m * NBF : (m + 1) * NBF],
                            emit_sb[pr, m * 128 : (m + 1) * 128],
                            O_sb[j][pr, :],
                            start=True,
                            stop=True,
                        )

                    if t == S - 1:
                        # beta starts at ones: w = e
                        w = wpool.tile([128, ZT], DT, name="wsb")
                        nc.scalar.copy(out=w[:], in_=e_ps[:])
                    else:
                        e_sb = epool.tile([128, ZT], DT, name="esb")
                        nc.scalar.copy(out=e_sb[:], in_=e_ps[:])
                        w = wpool.tile([128, ZT], DT, name="wsb")
                        nc.vector.tensor_mul(
                            out=w[:], in0=e_sb[:], in1=beta_ps[bc][:]
                        )

                    if t > 0:
                        for m in range(2):
                            for k in range(2):
                                nc.tensor.matmul(
                                    beta_ps[bc][:, m * NBF : (m + 1) * NBF],
                                    T_sb[k][:, m * 128 : (m + 1) * 128],
                                    w[:, k * NBF : (k + 1) * NBF],
                                    start=(k == 0),
                                    stop=(k == 1),
                                )
                    else:
                        s_ps = pse.tile([128, ZT], F32, name="eps")
                        for k in range(2):
                            nc.tensor.matmul(
                                s_ps[0:1, 0:NBF],
                                pi_sb[:, k : k + 1],
                                w[:, k * NBF : (k + 1) * NBF],
                                start=(k == 0),
                                stop=(k == 1),
                            )
                        s_sb = opool.tile([1, NBF], F32, name="ssb")
                        nc.vector.tensor_copy(out=s_sb[:], in_=s_ps[0:1, 0:NBF])
                        nc.sync.dma_start(
                            out=out_s[0:1, bc * NBF : (bc + 1) * NBF], in_=s_sb[:]
                        )

    nc.compile()
    return nc


def _get_nc():
    if "nc" not in _CACHE:
        _CACHE["nc"] = _build_bass()
    return _CACHE["nc"]


def _softmax0(x):
    x = np.asarray(x, np.float32)
    m = x.max(axis=0, keepdims=True)
    e = np.exp(x - m)
    return e / e.sum(axis=0, keepdims=True)


def _prepare_in_maps(tokens, T_logits, pi_logits, emit_logits):
    tokens = np.asarray(tokens).astype(np.int32)
    T = _softmax0(T_logits)                      # (Z, Z) columns sum to 1
    pi = _softmax0(pi_logits)                    # (Z,)
    emit = _softmax0(emit_logits) * np.float32(SCALE)  # (X, Z), pre-scaled

    Tb = T.astype(BF16)
    emit2 = np.concatenate([emit, emit], axis=0).astype(BF16)   # (128, Z)
    pi2 = pi.reshape(2, 128).T.copy().astype(BF16)              # (128, 2)

    # Packed one-hot: O[core, 64*(t%2)+token, (t//2)*BL + b_local] = 1
    O = np.zeros((NCORES, 128, (S // 2) * BL), np.float32)
    b = np.arange(B)
    core = b // BL
    bl = b % BL
    for t in range(S):
        O[core, 64 * (t % 2) + tokens[t], (t // 2) * BL + bl] = 1.0
    O = O.astype(BF16)

    return [
        {"O2": O[c], "Tm": Tb, "emit2": emit2, "pi2": pi2}
        for c in range(NCORES)
    ]


def _run(inputs, trace=False, tmpdir=None):
    from concourse.bass_utils import run_bass_kernel_spmd

    in_maps = _prepare_in_maps(
        inputs["tokens"],
        inputs["T_logits"],
        inputs["pi_logits"],
        inputs["emit_logits"],
    )
    nc = _get_nc()
    res = run_bass_kernel_spmd(
        nc, in_maps, list(range(NCORES)), trace=trace, tmpdir=tmpdir
    )
    s = np.concatenate(
        [res.results[c]["out_s"].reshape(-1) for c in range(NCORES)]
    ).astype(np.float32)
    out = np.float32(S * np.log(SCALE)) - np.log(s)
    return out.astype(np.float32), res


def kernel(**inputs):
    return _run(inputs, trace=False)[0]


# revision 3
# speedup vs baseline: 1.3237x; 1.0851x over previous
"""HMM forward-backward marginal (nn_HMM_EM) on 8 Trainium2 NeuronCores.

Batch (8192) is sharded across 8 cores (1024 each); tiny T/pi params are
replicated. The host precomputes the (scaled) emission softmax, gathers it by
token, and uploads per-core transposed emission tensors E (bf16). Per core,
in transposed (Z, B) layout with two pipelined 512-column batch chunks:
  recurse: beta^T <- T^T @ (e_t^T * beta^T)   11 steps, accumulated in PSUM
  reduce:  s = pi^T @ (e_0^T * beta^T)        (1, 1024) per core
Host post-processing: out = S*log(SCALE) - log(s).
Emissions are pre-scaled by SCALE=128 so all intermediates stay well inside
fp32 range (log s ~ -55 + 12*log 128 ~ +3).
"""

import sys

sys.path.insert(0, "/opt/trn_rl_repo")

import numpy as np
import ml_dtypes

Z = 256        # hidden states
X = 64         # emission symbols
S = 12         # sequence length
B = 8192       # total batch
NCORES = 8
BL = B // NCORES   # 1024 batch per core
NBF = 512          # matmul free-dim chunk (one PSUM bank of fp32)
NB = BL // NBF     # 2 batch chunks per core
SCALE = 128.0

BF16 = ml_dtypes.bfloat16

_CACHE: dict = {}


def _build_bass():
    import concourse.mybir as mybir
    from concourse import bacc
    from concourse.tile import TileContext

    DT = mybir.dt.bfloat16
    F32 = mybir.dt.float32

    nc = bacc.Bacc("TRN2", target_bir_lowering=False, debug=False)

    # E columns: ((t*NB + bc)*2 + m)*NBF + b, partition p = z % 128, m = z // 128
    E = nc.dram_tensor("E", [128, S * NB * 2 * NBF], DT, kind="ExternalInput")
    Tm = nc.dram_tensor("Tm", [Z, Z], DT, kind="ExternalInput")
    pi2 = nc.dram_tensor("pi2", [128, 2], DT, kind="ExternalInput")
    out_s = nc.dram_tensor("out_s", [1, BL], F32, kind="ExternalOutput")

    ZT = 2 * NBF  # 1024: two z-chunks side by side in the free dim

    with TileContext(nc) as tc:
        with (
            tc.tile_pool(name="const", bufs=1) as const,
            tc.tile_pool(name="bsb", bufs=3) as bpool,
            tc.tile_pool(name="wsb", bufs=4) as wpool,
            tc.tile_pool(name="osb", bufs=2) as opool,
            tc.tile_pool(name="pse", bufs=2, space="PSUM") as pse,
            tc.tile_pool(name="psb", bufs=1, space="PSUM") as psb,
        ):
            T_sb = [const.tile([128, Z], DT, name=f"T{k}") for k in range(2)]
            pi_sb = const.tile([128, 2], DT, name="pi2")
            E_sb = [const.tile([128, NB * ZT], DT, name=f"E{t}") for t in range(S)]

            # params on the gpsimd DMA queue so their issue overlaps E's
            nc.gpsimd.dma_start(out=pi_sb[:], in_=pi2[:])
            for k in range(2):
                nc.gpsimd.dma_start(
                    out=T_sb[k][:], in_=Tm[k * 128 : (k + 1) * 128, :]
                )
            # E chunks, issued in consumption order (t = S-1 first)
            for t in range(S - 1, -1, -1):
                nc.sync.dma_start(
                    out=E_sb[t][:], in_=E[:, t * NB * ZT : (t + 1) * NB * ZT]
                )

            # persistent 2-bank PSUM accumulators for beta^T, one per batch
            # chunk; columns [m*NBF,(m+1)*NBF) hold z-chunk m.
            beta_ps = [psb.tile([128, ZT], F32, name=f"beta{bc}") for bc in range(NB)]

            def e_slice(t, bc):
                return E_sb[t][:, bc * ZT : (bc + 1) * ZT]

            for t in range(S - 1, -1, -1):
                for bc in range(NB):
                    if t == S - 1:
                        w = e_slice(t, bc)  # beta starts at ones: w = e
                    elif bc == 0:
                        # multiply straight out of PSUM (1x mode)
                        wt = wpool.tile([128, ZT], DT, name="wsb")
                        nc.vector.tensor_mul(
                            out=wt[:], in0=e_slice(t, bc), in1=beta_ps[bc][:]
                        )
                        w = wt[:]
                    else:
                        # evacuate+cast beta on ScalarE, then 2x multiply
                        b_sb = bpool.tile([128, ZT], DT, name="bsb")
                        nc.scalar.copy(out=b_sb[:], in_=beta_ps[bc][:])
                        wt = wpool.tile([128, ZT], DT, name="wsb")
                        nc.vector.tensor_mul(
                            out=wt[:], in0=e_slice(t, bc), in1=b_sb[:]
                        )
                        w = wt[:]

                    if t > 0:
                        for m in range(2):
                            for k in range(2):
                                nc.tensor.matmul(
                                    beta_ps[bc][:, m * NBF : (m + 1) * NBF],
                                    T_sb[k][:, m * 128 : (m + 1) * 128],
                                    w[:, k * NBF : (k + 1) * NBF],
                                    start=(k == 0),
                                    stop=(k == 1),
                                )
                    else:
                        s_ps = pse.tile([128, NBF], F32, name="sps")
                        for k in range(2):
                            nc.tensor.matmul(
                                s_ps[0:1, :],
                                pi_sb[:, k : k + 1],
                                w[:, k * NBF : (k + 1) * NBF],
                                start=(k == 0),
                                stop=(k == 1),
                            )
                        s_sb = opool.tile([1, NBF], F32, name="ssb")
                        nc.vector.tensor_copy(out=s_sb[:], in_=s_ps[0:1, :])
                        nc.sync.dma_start(
                            out=out_s[0:1, bc * NBF : (bc + 1) * NBF], in_=s_sb[:]
                        )

    nc.compile()
    return nc


def _get_nc():
    if "nc" not in _CACHE:
        _CACHE["nc"] = _build_bass()
    return _CACHE["nc"]


def _softmax0(x):
    x = np.asarray(x, np.float32)
    m = x.max(axis=0, keepdims=True)
    e = np.exp(x - m)
    return e / e.sum(axis=0, keepdims=True)


def _prepare_in_maps(tokens, T_logits, pi_logits, emit_logits):
    tokens = np.asarray(tokens).astype(np.int32)
    T = _softmax0(T_logits)                      # (Z, Z) columns sum to 1
    pi = _softmax0(pi_logits)                    # (Z,)
    emit = _softmax0(emit_logits) * np.float32(SCALE)  # (X, Z), pre-scaled

    Tb = T.astype(BF16)
    pi2 = pi.reshape(2, 128).T.copy().astype(BF16)              # (128, 2)

    # Pre-gathered emissions, transposed per-core:
    # E[core][p, ((t*NB + bc)*2 + m)*NBF + b] = emit[tokens[t, g]] * SCALE
    # with g = core*BL + bc*NBF + b and z = m*128 + p.
    e_all = emit[tokens].astype(BF16)            # (S, B, Z)
    E = (
        e_all.reshape(S, NCORES, NB, NBF, 2, 128)
        .transpose(1, 5, 0, 2, 4, 3)
        .reshape(NCORES, 128, S * NB * 2 * NBF)
    )
    E = np.ascontiguousarray(E)

    return [{"E": E[c], "Tm": Tb, "pi2": pi2} for c in range(NCORES)]


def _run(inputs, trace=False, tmpdir=None):
    from concourse.bass_utils import run_bass_kernel_spmd

    in_maps = _prepare_in_maps(
        inputs["tokens"],
        inputs["T_logits"],
        inputs["pi_logits"],
        inputs["emit_logits"],
    )
    nc = _get_nc()
    res = run_bass_kernel_spmd(
        nc, in_maps, list(range(NCORES)), trace=trace, tmpdir=tmpdir
    )
    s = np.concatenate(
        [res.results[c]["out_s"].reshape(-1) for c in range(NCORES)]
    ).astype(np.float32)
    out = np.float32(S * np.log(SCALE)) - np.log(s)
    return out.astype(np.float32), res


def kernel(**inputs):
    return _run(inputs, trace=False)[0]


# revision 5
# speedup vs baseline: 1.5938x; 1.2040x over previous
"""HMM forward-backward marginal (nn_HMM_EM) on 8 Trainium2 NeuronCores.

Batch (8192) is sharded across 8 cores (1024 each); tiny T/pi params are
replicated. The host precomputes the (scaled) emission softmax, gathers it by
token, and uploads per-core transposed emission tensors E (bf16). Per core,
in transposed (Z, B) layout with two pipelined 512-column batch chunks:
  recurse: beta^T <- T^T @ (e_t^T * beta^T)   11 steps, accumulated in PSUM
  reduce:  s = pi^T @ (e_0^T * beta^T)        (1, 1024) per core
Host post-processing: out = S*log(SCALE) - log(s).
Emissions are pre-scaled by SCALE=128 so all intermediates stay well inside
fp32 range (log s ~ -55 + 12*log 128 ~ +3).
"""

import sys

sys.path.insert(0, "/opt/trn_rl_repo")

import numpy as np
import ml_dtypes

Z = 256        # hidden states
X = 64         # emission symbols
S = 12         # sequence length
B = 8192       # total batch
NCORES = 8
BL = B // NCORES   # 1024 batch per core
NBF = 512          # matmul free-dim chunk (one PSUM bank of fp32)
NB = BL // NBF     # 2 batch chunks per core
SCALE = 128.0

BF16 = ml_dtypes.bfloat16

_CACHE: dict = {}


def _build_bass():
    import concourse.mybir as mybir
    from concourse import bacc
    from concourse.tile import TileContext

    DT = mybir.dt.bfloat16
    F32 = mybir.dt.float32

    nc = bacc.Bacc("TRN2", target_bir_lowering=False, debug=False)

    # E columns: ((t*NB + bc)*2 + m)*NBF + b, partition p = z % 128, m = z // 128
    E = nc.dram_tensor("E", [128, S * NB * 2 * NBF], DT, kind="ExternalInput")
    Tm = nc.dram_tensor("Tm", [Z, Z], DT, kind="ExternalInput")
    pi2 = nc.dram_tensor("pi2", [128, 2], DT, kind="ExternalInput")
    out_s = nc.dram_tensor("out_s", [1, BL], F32, kind="ExternalOutput")

    ZT = 2 * NBF  # 1024: two z-chunks side by side in the free dim

    with TileContext(nc) as tc:
        with (
            tc.tile_pool(name="const", bufs=1) as const,
            tc.tile_pool(name="bsb", bufs=3) as bpool,
            tc.tile_pool(name="wsb", bufs=4) as wpool,
            tc.tile_pool(name="osb", bufs=2) as opool,
            tc.tile_pool(name="pse", bufs=2, space="PSUM") as pse,
            tc.tile_pool(name="psb", bufs=1, space="PSUM") as psb,
        ):
            T_sb = [const.tile([128, Z], DT, name=f"T{k}") for k in range(2)]
            pi_sb = const.tile([128, 2], DT, name="pi2")
            E_sb = [const.tile([128, NB * ZT], DT, name=f"E{t}") for t in range(S)]

            # params first (tiny, gate the first matmul), then E chunks in
            # consumption order (t = S-1 first), all on the sync HWDGE queue
            for k in range(2):
                nc.sync.dma_start(
                    out=T_sb[k][:], in_=Tm[k * 128 : (k + 1) * 128, :]
                )
            nc.sync.dma_start(out=pi_sb[:], in_=pi2[:])
            for t in range(S - 1, -1, -1):
                nc.sync.dma_start(
                    out=E_sb[t][:], in_=E[:, t * NB * ZT : (t + 1) * NB * ZT]
                )

            # persistent 2-bank PSUM accumulators for beta^T, one per batch
            # chunk; columns [m*NBF,(m+1)*NBF) hold z-chunk m.
            beta_ps = [psb.tile([128, ZT], F32, name=f"beta{bc}") for bc in range(NB)]

            def e_slice(t, bc):
                return E_sb[t][:, bc * ZT : (bc + 1) * ZT]

            for t in range(S - 1, -1, -1):
                for bc in range(NB):
                    if t == S - 1:
                        w = e_slice(t, bc)  # beta starts at ones: w = e
                    else:
                        # multiply straight out of PSUM (1x mode)
                        wt = wpool.tile([128, ZT], DT, name="wsb")
                        nc.vector.tensor_mul(
                            out=wt[:], in0=e_slice(t, bc), in1=beta_ps[bc][:]
                        )
                        w = wt[:]

                    if t > 0:
                        for m in range(2):
                            for k in range(2):
                                nc.tensor.matmul(
                                    beta_ps[bc][:, m * NBF : (m + 1) * NBF],
                                    T_sb[k][:, m * 128 : (m + 1) * 128],
                                    w[:, k * NBF : (k + 1) * NBF],
                                    start=(k == 0),
                                    stop=(k == 1),
                                )
                    else:
                        s_ps = pse.tile([128, NBF], F32, name="sps")
                        for k in range(2):
                            nc.tensor.matmul(
                                s_ps[0:1, :],
                                pi_sb[:, k : k + 1],
                                w[:, k * NBF : (k + 1) * NBF],
                                start=(k == 0),
                                stop=(k == 1),
                            )
                        s_sb = opool.tile([1, NBF], F32, name="ssb")
                        nc.vector.tensor_copy(out=s_sb[:], in_=s_ps[0:1, :])
                        nc.sync.dma_start(
                            out=out_s[0:1, bc * NBF : (bc + 1) * NBF], in_=s_sb[:]
                        )

    nc.compile()
    return nc


def _get_nc():
    if "nc" not in _CACHE:
        _CACHE["nc"] = _build_bass()
    return _CACHE["nc"]


def _softmax0(x):
    x = np.asarray(x, np.float32)
    m = x.max(axis=0, keepdims=True)
    e = np.exp(x - m)
    return e / e.sum(axis=0, keepdims=True)


def _prepare_in_maps(tokens, T_logits, pi_logits, emit_logits):
    tokens = np.asarray(tokens).astype(np.int32)
    T = _softmax0(T_logits)                      # (Z, Z) columns sum to 1
    pi = _softmax0(pi_logits)                    # (Z,)
    emit = _softmax0(emit_logits) * np.float32(SCALE)  # (X, Z), pre-scaled

    Tb = T.astype(BF16)
    pi2 = pi.reshape(2, 128).T.copy().astype(BF16)              # (128, 2)

    # Pre-gathered emissions, transposed per-core:
    # E[core][p, ((t*NB + bc)*2 + m)*NBF + b] = emit[tokens[t, g]] * SCALE
    # with g = core*BL + bc*NBF + b and z = m*128 + p.
    e_all = emit[tokens].astype(BF16)            # (S, B, Z)
    E = (
        e_all.reshape(S, NCORES, NB, NBF, 2, 128)
        .transpose(1, 5, 0, 2, 4, 3)
        .reshape(NCORES, 128, S * NB * 2 * NBF)
    )
    E = np.ascontiguousarray(E)

    return [{"E": E[c], "Tm": Tb, "pi2": pi2} for c in range(NCORES)]


def _run(inputs, trace=False, tmpdir=None):
    from concourse.bass_utils import run_bass_kernel_spmd

    in_maps = _prepare_in_maps(
        inputs["tokens"],
        inputs["T_logits"],
        inputs["pi_logits"],
        inputs["emit_logits"],
    )
    nc = _get_nc()
    res = run_bass_kernel_spmd(
        nc, in_maps, list(range(NCORES)), trace=trace, tmpdir=tmpdir
    )
    s = np.concatenate(
        [res.results[c]["out_s"].reshape(-1) for c in range(NCORES)]
    ).astype(np.float32)
    out = np.float32(S * np.log(SCALE)) - np.log(s)
    return out.astype(np.float32), res


def kernel(**inputs):
    return _run(inputs, trace=False)[0]
